# revision 21
# baseline (speedup 1.0000x reference)
"""Swin-style windowed multi-head cross-attention on 8 Trainium2 NeuronCores.

Full inputs in, full output out. Data-parallel over the window dim B_:
each of the 8 cores processes 256 windows end-to-end (no collectives).

Per-core Bass/Tile program, software-pipelined across 4-window "quads" so
the PE never idles long enough for the HAM clock-gate to re-throttle:

  iteration i emits:  loads(i+1) | QK(i) | exp+bias-mul(i) | proj(i+1) |
                      AV(i) | divide(i) | transpose+out-proj(i-1)

Algorithm per quad (4 windows, T=2 memory frames, 8 heads, hd=32):
  1. q/k projections in transposed orientation (qT/kT channel-major),
     v projection natural; v copied into persistent vn tiles with a
     per-head ones-column (33-col groups) that later yields the softmax
     denominators for free inside the AV matmul.
  2. QK^T per (window, hh, rg) into 4 PSUM banks grouped by PE row-group
     (matmuls sharing a bank must share their array row position).
  3. exp on ACT; then exp(S)*exp(B) is written MASKED into persistent
     block-diagonal E3 tiles [m, (w, hh, frame, n)]: frame-f columns only
     get rows of frame f, other rows stay zero (written once at init).
     This both applies the relative-position bias and builds the
     frame-separating block structure for AV in one DVE pass.
  4. AV: one matmul per (window, head): lhsT=E3-slice [128,128] (FWL),
     rhs=vn 33-col slice; out av[frame*64+n, (w%2)*264+h*33+{d|den}] in
     2 bf16 PSUM banks. Ones columns give denominators.
  5. reciprocal + one broadcast-divide per av bank -> ad (token-major);
     transpose back to channel-major via plain matmuls against identity
     (faster than PE transpose-mode and HAM-friendly); final projection;
     bf16 store with host-side f32 upcast.

Softmax max-subtraction is skipped: logits are ~N(0,1)+-small bias, exp is
safe in f32/bf16 and the result is mathematically identical.
q bias is applied on-device (per-partition ACT bias) only when nonzero;
k bias is a per-query constant in the logits (softmax-invariant) so it is
dropped exactly; v bias and proj bias fold into a host-side output add.
"""

import numpy as np
import ml_dtypes

import concourse.bacc as bacc
import concourse.mybir as mybir
import concourse.tile as tile
from concourse.bass_utils import run_bass_kernel_spmd

PH = PW = 8
N = 64          # tokens per window
C = 256         # channels
H = 8           # heads
HD = 32         # head dim
T = 2           # memory frames per window
NCORES = 8
B_FULL = 2048
B_CORE = B_FULL // NCORES
QW = 4          # windows per block
BF16 = ml_dtypes.bfloat16

F32 = mybir.dt.float32
BF = mybir.dt.bfloat16
AF = mybir.ActivationFunctionType


def _rel_idx(ph, pw):
    coords = np.stack(np.meshgrid(np.arange(ph), np.arange(pw), indexing="ij"))
    flat = coords.reshape(2, -1)
    rel = flat[:, :, None] - flat[:, None, :]
    rel = rel.transpose(1, 2, 0).copy()
    rel[:, :, 0] += ph - 1
    rel[:, :, 1] += pw - 1
    rel[:, :, 0] *= 2 * pw - 1
    return rel.sum(-1)  # (N, N) int


REL_IDX = _rel_idx(PH, PW)

_PROG_CACHE = {}


def _build(nwin, with_qbias):
    """Build the per-core Bass program for nwin windows."""
    assert nwin % QW == 0
    nq = nwin // QW
    xt = nwin * N            # x tokens
    mt = nwin * T * N        # memory tokens

    nc = bacc.Bacc("TRN2", target_bir_lowering=False)

    x_d = nc.dram_tensor("xT", [C, xt], BF, kind="ExternalInput").ap()
    m_d = nc.dram_tensor("memT", [C, mt], BF, kind="ExternalInput").ap()
    qw_d = nc.dram_tensor("qwT", [C, C], BF, kind="ExternalInput").ap()
    kw_d = nc.dram_tensor("kwT", [C, C], BF, kind="ExternalInput").ap()
    vw_d = nc.dram_tensor("vwT", [C, C], BF, kind="ExternalInput").ap()
    pw_d = nc.dram_tensor("pwT", [C, C], BF, kind="ExternalInput").ap()
    eb_d = nc.dram_tensor("expBT", [128, 2048], BF, kind="ExternalInput").ap()
    id_d = nc.dram_tensor("idn", [128, 128], BF, kind="ExternalInput").ap()
    if with_qbias:
        qb_d = nc.dram_tensor("qbT", [128, 2], F32, kind="ExternalInput").ap()
    out_d = nc.dram_tensor("out", [mt, C], BF, kind="ExternalOutput").ap()

    from contextlib import ExitStack
    with ExitStack() as ctx:
        tc = ctx.enter_context(tile.TileContext(nc, pool_alloc_mode="queue"))
        const = ctx.enter_context(tc.tile_pool(name="const", bufs=1))
        sb = ctx.enter_context(tc.tile_pool(name="sb", bufs=2))
        ps = ctx.enter_context(tc.tile_pool(name="ps", bufs=8, space="PSUM"))

        # ---- constants (each weight: [128, 2*C], K-chunk kc at cols kc*C) ----
        qwT_s = const.tile([128, 2 * C], BF, tag="qwT", name="qwT_s")
        kwT_s = const.tile([128, 2 * C], BF, tag="kwT", name="kwT_s")
        vwT_s = const.tile([128, 2 * C], BF, tag="vwT", name="vwT_s")
        pwT_s = const.tile([128, 2 * C], BF, tag="pwT", name="pwT_s")
        for kc in range(2):
            nc.sync.dma_start(out=qwT_s[:, kc * C:(kc + 1) * C], in_=qw_d[kc * 128:(kc + 1) * 128, :])
            nc.sync.dma_start(out=kwT_s[:, kc * C:(kc + 1) * C], in_=kw_d[kc * 128:(kc + 1) * 128, :])
            nc.sync.dma_start(out=vwT_s[:, kc * C:(kc + 1) * C], in_=vw_d[kc * 128:(kc + 1) * 128, :])
            nc.sync.dma_start(out=pwT_s[:, kc * C:(kc + 1) * C], in_=pw_d[kc * 128:(kc + 1) * 128, :])
        qwT = [qwT_s[:, 0:C], qwT_s[:, C:2 * C]]
        kwT = [kwT_s[:, 0:C], kwT_s[:, C:2 * C]]
        vwT = [vwT_s[:, 0:C], vwT_s[:, C:2 * C]]
        pwT = [pwT_s[:, 0:C], pwT_s[:, C:2 * C]]
        expB = const.tile([128, 2048], BF, tag="expB", name="expB")
        nc.sync.dma_start(out=expB[:], in_=eb_d[:, :])
        idn = const.tile([128, 128], BF, tag="idn", name="idn")
        nc.sync.dma_start(out=idn[:], in_=id_d[:, :])
        if with_qbias:
            qbT = const.tile([128, 2], F32, tag="qbT", name="qbT")
            nc.sync.dma_start(out=qbT[:], in_=qb_d[:, :])

        # persistent masked-E tiles: E3[rg][m, w*256 + hh*128 + f*64 + n]
        # = exp(S+B) if frame(m)==f else 0.  Zero blocks written once here.
        E3 = [const.tile([128, 1024], BF, tag=f"E3_{rg}", name="E3")
              for rg in range(4)]
        for e in E3:
            nc.vector.memset(e[:], 0.0)

        # persistent vn tiles with fused denominator ones-columns:
        # vn[(q%2)*2+b][m, tl*264 + h*33 + d] = v, col h*33+32 = 1.
        vn_all = [const.tile([128, 528], BF, tag=f"vn{i}", name="vn")
                  for i in range(4)]
        for v in vn_all:
            nc.vector.memset(
                v[:, :].rearrange("p (x c) -> p x c", c=33)[:, :, 32:33], 1.0)

        st = [{} for _ in range(nq)]  # per-quad tile handles

        def emit_loads(q):
            xr = q * QW * N
            mr = q * QW * T * N
            xT = sb.tile([128, 512], BF, tag="xT", name="xT", bufs=3)
            for cc in range(2):
                nc.sync.dma_start(out=xT[:, cc * 256:(cc + 1) * 256],
                                  in_=x_d[cc * 128:(cc + 1) * 128, xr:xr + 256])
            memT = []
            for cc in range(2):
                mT = sb.tile([128, 512], BF, tag="memT", name="memT", bufs=6)
                nc.sync.dma_start(out=mT[:], in_=m_d[cc * 128:(cc + 1) * 128, mr:mr + 512])
                memT.append(mT)
            st[q]["xT"] = xT
            st[q]["memT"] = memT

        def emit_proj(q):
            xT = st[q]["xT"]
            memT = st[q]["memT"]
            # ---- q projection (transposed: qT[c_out%128, hh*256 + w*64+n]) ----
            pq = ps.tile([128, 512], F32, tag="ps", name="ps")
            for mo in range(2):
                for kc in range(2):
                    nc.tensor.matmul(
                        pq[:, mo * 256:(mo + 1) * 256],
                        lhsT=qwT[kc][:, mo * 128:(mo + 1) * 128],
                        rhs=xT[:, kc * 256:(kc + 1) * 256],
                        start=(kc == 0), stop=(kc == 1))
            qT = sb.tile([128, 512], BF, tag="qT", name="qT", bufs=3)
            if with_qbias:
                for mo in range(2):
                    nc.scalar.activation(qT[:, mo * 256:(mo + 1) * 256],
                                         pq[:, mo * 256:(mo + 1) * 256],
                                         AF.Copy, bias=qbT[:, mo:mo + 1])
            else:
                nc.scalar.copy(qT[:], pq[:])
            st[q]["qT"] = qT

            # ---- k projection (transposed: kT[mo][c, w*128 + t*64+n]) ----
            kT = []
            for mo in range(2):
                pk = ps.tile([128, 512], F32, tag="ps", name="ps")
                for kc in range(2):
                    nc.tensor.matmul(
                        pk[:],
                        lhsT=kwT[kc][:, mo * 128:(mo + 1) * 128],
                        rhs=memT[kc][:],
                        start=(kc == 0), stop=(kc == 1))
                kt = sb.tile([128, 512], BF, tag="kT", name="kT", bufs=4)
                if mo == 0:
                    nc.scalar.copy(kt[:], pk[:])
                else:
                    nc.vector.tensor_copy(kt[:], pk[:])
                kT.append(kt)
            st[q]["kT"] = kT

            # ---- v projection (natural) -> persistent vn (33-col groups) ----
            vns = []
            for b in range(2):
                pv = ps.tile([128, 512], F32, tag="ps", name="ps")
                for tl in range(2):
                    tt = b * 2 + tl
                    for kc in range(2):
                        nc.tensor.matmul(
                            pv[:, tl * 256:(tl + 1) * 256],
                            lhsT=memT[kc][:, tt * 128:(tt + 1) * 128],
                            rhs=vwT[kc][:],
                            start=(kc == 0), stop=(kc == 1))
                vt = vn_all[(q % 2) * 2 + b]
                nc.scalar.copy(
                    vt[:, :].rearrange("p (tl h c) -> p tl h c", tl=2, c=33)
                        [:, :, :, 0:32],
                    pv[:, :].rearrange("p (tl h d) -> p tl h d", tl=2, d=32))
                vns.append(vt)
            st[q]["vn"] = vns

        def emit_qk(q):
            qT = st[q]["qT"]
            kT = st[q]["kT"]
            # Bank rg holds heads {rg, rg+4}: psS[rg][m, w*128 + hh*64 + n].
            psS = [ps.tile([128, 512], F32, tag="ps", name="ps") for _ in range(4)]
            for hh in range(2):
                for w in range(QW):
                    for rg in range(4):
                        nc.tensor.matmul(
                            psS[rg][:, w * 128 + hh * 64: w * 128 + (hh + 1) * 64],
                            lhsT=kT[hh][rg * 32:(rg + 1) * 32, w * 128:(w + 1) * 128],
                            rhs=qT[rg * 32:(rg + 1) * 32,
                                   hh * 256 + w * 64: hh * 256 + (w + 1) * 64],
                            start=True, stop=True,
                            tile_position=(rg * 32, 0))
            st[q]["psS"] = psS

        def emit_softmax(q):
            # exp on ACT into one wide E tile, then two masked bias-multiply
            # ops write the block-diagonal E3 (g = 32 (rg,w,hh) blocks).
            psS = st[q]["psS"]
            for rg in range(4):
                Ew = sb.tile([128, 512], BF, tag="E", name="E", bufs=4)
                nc.scalar.activation(Ew[:], psS[rg][:], AF.Exp)
                for f in range(2):
                    rows = slice(f * 64, (f + 1) * 64)
                    nc.vector.tensor_mul(
                        E3[rg][rows, :].rearrange(
                            "p (w hh f2 n) -> p w hh f2 n", w=4, hh=2, f2=2)
                            [:, :, :, f, :],
                        Ew[rows, :].rearrange(
                            "p (w hh n) -> p w hh n", w=4, hh=2),
                        expB[rows, rg * 512:(rg + 1) * 512].rearrange(
                            "p (w hh n) -> p w hh n", w=4, hh=2))

        def emit_av(q):
            vns = st[q]["vn"]
            # av[w][f*64+n, h*33 + {d|den}], f32, one bank per window.
            av = []
            for w in range(QW):
                avw = ps.tile([128, 264], F32, tag="ps", name="ps")
                for h in range(H):
                    cb = w * 256 + (h // 4) * 128
                    nc.tensor.matmul(
                        avw[:, h * 33:(h + 1) * 33],
                        lhsT=E3[h % 4][:, cb:cb + 128],
                        rhs=vns[w // 2][:, (w % 2) * 264 + h * 33:
                                        (w % 2) * 264 + (h + 1) * 33],
                        start=True, stop=True)
                av.append(avw)
            st[q]["av"] = av

        def emit_div(q):
            av = st[q]["av"]
            rc = sb.tile([128, 32], F32, tag="rc", name="rc", bufs=2)
            rc2 = sb.tile([128, 32], F32, tag="rc2", name="rc2", bufs=2)
            # ad[w][f*64+n, h*32 + d] (token-major, bf16); per-window chains
            # so div(w) overlaps the later windows' AV matmuls.
            ad = []
            for w in range(QW):
                nc.scalar.copy(
                    rc[:, w * 8:(w + 1) * 8],
                    av[w][:, :].rearrange("p (x c) -> p x c", c=33)[:, :, 32])
                nc.vector.reciprocal(rc2[:, w * 8:(w + 1) * 8],
                                     rc[:, w * 8:(w + 1) * 8])
                adw = sb.tile([128, 256], BF, tag="ad", name="ad", bufs=8)
                nc.vector.tensor_mul(
                    adw[:, :].rearrange("p (h d) -> p h d", d=32),
                    av[w][:, :].rearrange("p (h c) -> p h c", c=33)[:, :, 0:32],
                    rc2[:, w * 8:(w + 1) * 8].broadcast_to((128, 8, 32)))
                ad.append(adw)
            st[q]["ad"] = ad

        def emit_tail(q):
            mr = q * QW * T * N
            ad = st[q]["ad"]
            # transpose token-major -> channel-major via matmul against I.
            pot = [ps.tile([128, 512], F32, tag="ps", name="ps")
                   for _ in range(2)]
            for cc in range(2):
                for w in range(QW):
                    nc.tensor.matmul(
                        pot[cc][:, w * 128:(w + 1) * 128],
                        lhsT=ad[w][:, cc * 128:(cc + 1) * 128],
                        rhs=idn[:],
                        start=True, stop=True)
            aT = []
            for cc in range(2):
                aTc = sb.tile([128, 512], BF, tag="aT", name="aT", bufs=4)
                if cc == 0:
                    nc.vector.tensor_copy(aTc[:], pot[cc][:])
                else:
                    nc.scalar.copy(aTc[:], pot[cc][:])
                aT.append(aTc)

            # final projection: py[b2][f*64+n, (w%2)*256 + c], f32.
            py = [ps.tile([128, 512], F32, tag="ps", name="ps")
                  for _ in range(2)]
            for w in range(QW):
                for cc in range(2):
                    nc.tensor.matmul(
                        py[w // 2][:, (w % 2) * 256:(w % 2 + 1) * 256],
                        lhsT=aT[cc][:, w * 128:(w + 1) * 128],
                        rhs=pwT[cc][:],
                        start=(cc == 0), stop=(cc == 1))
            for j in range(2):
                yo = sb.tile([128, 512], BF, tag="yo", name="yo", bufs=4)
                if j == 0:
                    nc.scalar.copy(yo[:], py[j][:])
                else:
                    nc.vector.tensor_copy(yo[:], py[j][:])
                for w2 in range(2):
                    base = mr + j * 256 + w2 * 128
                    nc.gpsimd.dma_start(
                        out=out_d[base: base + 128, :],
                        in_=yo[:, w2 * 256:(w2 + 1) * 256])

        # ---- software-pipelined main loop ----
        emit_loads(0)
        emit_proj(0)
        for i in range(nq):
            if i + 1 < nq:
                emit_loads(i + 1)
            emit_qk(i)
            emit_softmax(i)
            if i + 1 < nq:
                emit_proj(i + 1)
            if i > 0:
                emit_tail(i - 1)
            emit_av(i)
            emit_div(i)
            # drop handles no longer needed to keep memory flat
            st[i]["psS"] = None
            if i > 0:
                st[i - 1] = None
        emit_tail(nq - 1)

    nc.compile()
    return nc


def _get_prog(nwin, with_qbias):
    key = (nwin, with_qbias)
    if key not in _PROG_CACHE:
        _PROG_CACHE[key] = _build(nwin, with_qbias)
    return _PROG_CACHE[key]


def _host_prep(x, memory, q_w, q_b, kv_w, kv_b, proj_w, proj_b, rpb_table,
               nwin_core, ncores):
    """Shared host-side preprocessing: fold scales/biases, shard, cast."""
    x = np.asarray(x, np.float32)
    memory = np.asarray(memory, np.float32)
    q_w = np.asarray(q_w, np.float32)
    q_b = np.asarray(q_b, np.float32)
    kv_w = np.asarray(kv_w, np.float32)
    kv_b = np.asarray(kv_b, np.float32)
    proj_w = np.asarray(proj_w, np.float32)
    proj_b = np.asarray(proj_b, np.float32)
    rpb_table = np.asarray(rpb_table, np.float32)

    scale = np.float32(HD ** -0.5)
    qwT = np.ascontiguousarray((q_w * scale).T).astype(BF16)       # [c_in, c_out]
    kwT = np.ascontiguousarray(kv_w[:C].T).astype(BF16)
    vwT = np.ascontiguousarray(kv_w[C:].T).astype(BF16)
    pwT = np.ascontiguousarray(proj_w.T).astype(BF16)

    # logits bias, transposed per head: biasT[h][m, n]; exp() folded in.
    # Device layout [128, 2048]: row t*64+m (both t halves equal),
    # col rg*512 + w*128 + hh*64 + n, for head = rg + 4*hh (w-invariant).
    bias = rpb_table[REL_IDX.reshape(-1)].reshape(N, N, H)          # [n, m, h]
    biasT = bias.transpose(2, 1, 0)                                 # [h, m, n]
    blocks = []
    for rg in range(4):
        pair = np.concatenate([biasT[rg], biasT[rg + 4]], axis=1)   # [64, 128]
        blocks.append(np.tile(pair, (1, QW)))                       # [64, 512]
    half = np.concatenate(blocks, axis=1)                           # [64, 2048]
    expBT = np.exp(np.vstack([half, half])).astype(BF16)            # [128, 2048]

    idn = np.eye(128, dtype=BF16)

    qb_eff = q_b * scale
    with_qbias = bool(np.any(qb_eff))
    qbT = np.ascontiguousarray(qb_eff.reshape(2, 128).T).astype(np.float32)

    # k bias: constant per query in logits -> softmax-invariant, dropped.
    # v bias + proj bias: fold into a host-side output add.
    out_bias = kv_b[C:] @ proj_w.T + proj_b                          # (C,)

    # host-side transpose to channel-major (device loads these directly)
    xs = x.reshape(B_FULL * N, C).astype(BF16)
    ms = memory.reshape(B_FULL * T * N, C).astype(BF16)

    xt = nwin_core * N
    mt = nwin_core * T * N
    in_maps = []
    for c in range(ncores):
        im = {
            "xT": np.ascontiguousarray(xs[c * xt:(c + 1) * xt].T),
            "memT": np.ascontiguousarray(ms[c * mt:(c + 1) * mt].T),
            "qwT": qwT, "kwT": kwT, "vwT": vwT, "pwT": pwT,
            "expBT": expBT, "idn": idn,
        }
        if with_qbias:
            im["qbT"] = qbT
        in_maps.append(im)
    return in_maps, with_qbias, out_bias


def kernel(x, memory, q_w, q_b, kv_w, kv_b, proj_w, proj_b, rpb_table):
    in_maps, with_qbias, out_bias = _host_prep(
        x, memory, q_w, q_b, kv_w, kv_b, proj_w, proj_b, rpb_table,
        B_CORE, NCORES)
    nc = _get_prog(B_CORE, with_qbias)
    res = run_bass_kernel_spmd(nc, in_maps, list(range(NCORES)))
    out = np.concatenate([np.asarray(r["out"]) for r in res.results], axis=0)
    out = out.astype(np.float32)
    if np.any(out_bias):
        out = out + out_bias[None, :].astype(np.float32)
    return np.ascontiguousarray(out.reshape(B_FULL, T, N, C))


# revision 23
# speedup vs baseline: 1.1535x; 1.1535x over previous
"""Swin-style windowed multi-head cross-attention on 8 Trainium2 NeuronCores.

Full inputs in, full output out. Data-parallel over the window dim B_:
each of the 8 cores processes 256 windows end-to-end (no collectives).

Per-core Bass/Tile program, software-pipelined across 4-window "quads" so
the PE never idles long enough for the HAM clock-gate to re-throttle:

  iteration i emits:  loads(i+1) | QK(i) | exp+bias-mul(i) | proj(i+1) |
                      AV(i) | divide(i) | transpose+out-proj(i-1)

Algorithm per quad (4 windows, T=2 memory frames, 8 heads, hd=32):
  1. q/k projections in transposed orientation (qT/kT channel-major),
     v projection natural; v copied into persistent vn tiles with a
     per-head ones-column (33-col groups) that later yields the softmax
     denominators for free inside the AV matmul.
  2. QK^T per (window, hh, rg) into 4 PSUM banks grouped by PE row-group
     (matmuls sharing a bank must share their array row position).
  3. exp on ACT; then exp(S)*exp(B) is written MASKED into persistent
     block-diagonal E3 tiles [m, (w, hh, frame, n)]: frame-f columns only
     get rows of frame f, other rows stay zero (written once at init).
     This both applies the relative-position bias and builds the
     frame-separating block structure for AV in one DVE pass.
  4. AV: one matmul per (window, head): lhsT=E3-slice [128,128] (FWL),
     rhs=vn 33-col slice; out av[frame*64+n, (w%2)*264+h*33+{d|den}] in
     2 bf16 PSUM banks. Ones columns give denominators.
  5. reciprocal + one broadcast-divide per av bank -> ad (token-major);
     transpose back to channel-major via plain matmuls against identity
     (faster than PE transpose-mode and HAM-friendly); final projection;
     bf16 store with host-side f32 upcast.

Softmax max-subtraction is skipped: logits are ~N(0,1)+-small bias, exp is
safe in f32/bf16 and the result is mathematically identical.
q bias is applied on-device (per-partition ACT bias) only when nonzero;
k bias is a per-query constant in the logits (softmax-invariant) so it is
dropped exactly; v bias and proj bias fold into a host-side output add.
"""

import numpy as np
import ml_dtypes

import concourse.bacc as bacc
import concourse.mybir as mybir
import concourse.tile as tile
from concourse.bass_utils import run_bass_kernel_spmd

PH = PW = 8
N = 64          # tokens per window
C = 256         # channels
H = 8           # heads
HD = 32         # head dim
T = 2           # memory frames per window
NCORES = 8
B_FULL = 2048
B_CORE = B_FULL // NCORES
QW = 4          # windows per block
BF16 = ml_dtypes.bfloat16

F32 = mybir.dt.float32
BF = mybir.dt.bfloat16
AF = mybir.ActivationFunctionType


def _rel_idx(ph, pw):
    coords = np.stack(np.meshgrid(np.arange(ph), np.arange(pw), indexing="ij"))
    flat = coords.reshape(2, -1)
    rel = flat[:, :, None] - flat[:, None, :]
    rel = rel.transpose(1, 2, 0).copy()
    rel[:, :, 0] += ph - 1
    rel[:, :, 1] += pw - 1
    rel[:, :, 0] *= 2 * pw - 1
    return rel.sum(-1)  # (N, N) int


REL_IDX = _rel_idx(PH, PW)

_PROG_CACHE = {}


def _build(nwin, with_qbias):
    """Build the per-core Bass program for nwin windows."""
    assert nwin % QW == 0
    nq = nwin // QW
    xt = nwin * N            # x tokens
    mt = nwin * T * N        # memory tokens

    nc = bacc.Bacc("TRN2", target_bir_lowering=False)

    x_d = nc.dram_tensor("xT", [C, xt], BF, kind="ExternalInput").ap()
    m_d = nc.dram_tensor("memT", [C, mt], BF, kind="ExternalInput").ap()
    qw_d = nc.dram_tensor("qwT", [C, C], BF, kind="ExternalInput").ap()
    kw_d = nc.dram_tensor("kwT", [C, C], BF, kind="ExternalInput").ap()
    vw_d = nc.dram_tensor("vwT", [C, C], BF, kind="ExternalInput").ap()
    pw_d = nc.dram_tensor("pwT", [C, C], BF, kind="ExternalInput").ap()
    eb_d = nc.dram_tensor("expBT", [128, 2048], BF, kind="ExternalInput").ap()
    id_d = nc.dram_tensor("idn", [128, 128], BF, kind="ExternalInput").ap()
    if with_qbias:
        qb_d = nc.dram_tensor("qbT", [128, 2], F32, kind="ExternalInput").ap()
    out_d = nc.dram_tensor("out", [mt, C], BF, kind="ExternalOutput").ap()

    from contextlib import ExitStack
    with ExitStack() as ctx:
        tc = ctx.enter_context(tile.TileContext(nc, pool_alloc_mode="queue"))
        const = ctx.enter_context(tc.tile_pool(name="const", bufs=1))
        sb = ctx.enter_context(tc.tile_pool(name="sb", bufs=2))
        ps = ctx.enter_context(tc.tile_pool(name="ps", bufs=8, space="PSUM"))

        # ---- constants (each weight: [128, 2*C], K-chunk kc at cols kc*C) ----
        qwT_s = const.tile([128, 2 * C], BF, tag="qwT", name="qwT_s")
        kwT_s = const.tile([128, 2 * C], BF, tag="kwT", name="kwT_s")
        vwT_s = const.tile([128, 2 * C], BF, tag="vwT", name="vwT_s")
        pwT_s = const.tile([128, 2 * C], BF, tag="pwT", name="pwT_s")
        for kc in range(2):
            nc.sync.dma_start(out=qwT_s[:, kc * C:(kc + 1) * C], in_=qw_d[kc * 128:(kc + 1) * 128, :])
            nc.sync.dma_start(out=kwT_s[:, kc * C:(kc + 1) * C], in_=kw_d[kc * 128:(kc + 1) * 128, :])
            nc.sync.dma_start(out=vwT_s[:, kc * C:(kc + 1) * C], in_=vw_d[kc * 128:(kc + 1) * 128, :])
            nc.sync.dma_start(out=pwT_s[:, kc * C:(kc + 1) * C], in_=pw_d[kc * 128:(kc + 1) * 128, :])
        qwT = [qwT_s[:, 0:C], qwT_s[:, C:2 * C]]
        kwT = [kwT_s[:, 0:C], kwT_s[:, C:2 * C]]
        vwT = [vwT_s[:, 0:C], vwT_s[:, C:2 * C]]
        pwT = [pwT_s[:, 0:C], pwT_s[:, C:2 * C]]
        expB = const.tile([128, 2048], BF, tag="expB", name="expB")
        nc.sync.dma_start(out=expB[:], in_=eb_d[:, :])
        idn = const.tile([128, 128], BF, tag="idn", name="idn")
        nc.sync.dma_start(out=idn[:], in_=id_d[:, :])
        if with_qbias:
            qbT = const.tile([128, 2], F32, tag="qbT", name="qbT")
            nc.sync.dma_start(out=qbT[:], in_=qb_d[:, :])

        # persistent masked-E tiles: E3[rg][m, w*256 + hh*128 + f*64 + n]
        # = exp(S+B) if frame(m)==f else 0.  Zero blocks written once here.
        E3 = [const.tile([128, 1024], BF, tag=f"E3_{rg}", name="E3")
              for rg in range(4)]
        for e in E3:
            nc.vector.memset(e[:], 0.0)

        # persistent vn tiles with fused denominator ones-columns:
        # vn[(q%2)*2+b][m, tl*264 + h*33 + d] = v, col h*33+32 = 1.
        vn_all = [const.tile([128, 528], BF, tag=f"vn{i}", name="vn")
                  for i in range(4)]
        for v in vn_all:
            nc.vector.memset(
                v[:, :].rearrange("p (x c) -> p x c", c=33)[:, :, 32:33], 1.0)

        st = [{} for _ in range(nq)]  # per-quad tile handles

        def emit_loads(q):
            xr = q * QW * N
            mr = q * QW * T * N
            xT = sb.tile([128, 512], BF, tag="xT", name="xT", bufs=3)
            for cc in range(2):
                nc.sync.dma_start(out=xT[:, cc * 256:(cc + 1) * 256],
                                  in_=x_d[cc * 128:(cc + 1) * 128, xr:xr + 256])
            memT = []
            for cc in range(2):
                mT = sb.tile([128, 512], BF, tag="memT", name="memT", bufs=6)
                nc.sync.dma_start(out=mT[:], in_=m_d[cc * 128:(cc + 1) * 128, mr:mr + 512])
                memT.append(mT)
            st[q]["xT"] = xT
            st[q]["memT"] = memT

        def emit_proj(q):
            xT = st[q]["xT"]
            memT = st[q]["memT"]
            # ---- q projection (transposed: qT[c_out%128, hh*256 + w*64+n]) ----
            pq = ps.tile([128, 512], F32, tag="ps", name="ps")
            for mo in range(2):
                for kc in range(2):
                    nc.tensor.matmul(
                        pq[:, mo * 256:(mo + 1) * 256],
                        lhsT=qwT[kc][:, mo * 128:(mo + 1) * 128],
                        rhs=xT[:, kc * 256:(kc + 1) * 256],
                        start=(kc == 0), stop=(kc == 1))
            qT = sb.tile([128, 512], BF, tag="qT", name="qT", bufs=3)
            if with_qbias:
                for mo in range(2):
                    nc.scalar.activation(qT[:, mo * 256:(mo + 1) * 256],
                                         pq[:, mo * 256:(mo + 1) * 256],
                                         AF.Copy, bias=qbT[:, mo:mo + 1])
            else:
                nc.scalar.copy(qT[:], pq[:])
            st[q]["qT"] = qT

            # ---- k projection (transposed: kT[mo][c, w*128 + t*64+n]) ----
            kT = []
            for mo in range(2):
                pk = ps.tile([128, 512], F32, tag="ps", name="ps")
                for kc in range(2):
                    nc.tensor.matmul(
                        pk[:],
                        lhsT=kwT[kc][:, mo * 128:(mo + 1) * 128],
                        rhs=memT[kc][:],
                        start=(kc == 0), stop=(kc == 1))
                kt = sb.tile([128, 512], BF, tag="kT", name="kT", bufs=4)
                if mo == 0:
                    nc.scalar.copy(kt[:], pk[:])
                else:
                    nc.vector.tensor_copy(kt[:], pk[:])
                kT.append(kt)
            st[q]["kT"] = kT

            # ---- v projection (natural) -> persistent vn (33-col groups) ----
            vns = []
            for b in range(2):
                pv = ps.tile([128, 512], F32, tag="ps", name="ps")
                for tl in range(2):
                    tt = b * 2 + tl
                    for kc in range(2):
                        nc.tensor.matmul(
                            pv[:, tl * 256:(tl + 1) * 256],
                            lhsT=memT[kc][:, tt * 128:(tt + 1) * 128],
                            rhs=vwT[kc][:],
                            start=(kc == 0), stop=(kc == 1))
                vt = vn_all[(q % 2) * 2 + b]
                nc.scalar.copy(
                    vt[:, :].rearrange("p (tl h c) -> p tl h c", tl=2, c=33)
                        [:, :, :, 0:32],
                    pv[:, :].rearrange("p (tl h d) -> p tl h d", tl=2, d=32))
                vns.append(vt)
            st[q]["vn"] = vns

        def emit_qk(q):
            qT = st[q]["qT"]
            kT = st[q]["kT"]
            # Bank rg holds heads {rg, rg+4}: psS[rg][m, w*128 + hh*64 + n].
            psS = [ps.tile([128, 512], F32, tag="ps", name="ps") for _ in range(4)]
            for hh in range(2):
                for w in range(QW):
                    for rg in range(4):
                        nc.tensor.matmul(
                            psS[rg][:, w * 128 + hh * 64: w * 128 + (hh + 1) * 64],
                            lhsT=kT[hh][rg * 32:(rg + 1) * 32, w * 128:(w + 1) * 128],
                            rhs=qT[rg * 32:(rg + 1) * 32,
                                   hh * 256 + w * 64: hh * 256 + (w + 1) * 64],
                            start=True, stop=True,
                            tile_position=(rg * 32, 0))
            st[q]["psS"] = psS

        def emit_softmax(q):
            # exp on ACT into one wide E tile, then two masked bias-multiply
            # ops write the block-diagonal E3 (g = 32 (rg,w,hh) blocks).
            psS = st[q]["psS"]
            for rg in range(4):
                Ew = sb.tile([128, 512], BF, tag="E", name="E", bufs=4)
                nc.scalar.activation(Ew[:], psS[rg][:], AF.Exp)
                for f in range(2):
                    rows = slice(f * 64, (f + 1) * 64)
                    nc.vector.tensor_mul(
                        E3[rg][rows, :].rearrange(
                            "p (w hh f2 n) -> p w hh f2 n", w=4, hh=2, f2=2)
                            [:, :, :, f, :],
                        Ew[rows, :].rearrange(
                            "p (w hh n) -> p w hh n", w=4, hh=2),
                        expB[rows, rg * 512:(rg + 1) * 512].rearrange(
                            "p (w hh n) -> p w hh n", w=4, hh=2))

        def emit_av(q):
            vns = st[q]["vn"]
            # av[w][f*64+n, h*33 + {d|den}], f32, one bank per window.
            av = []
            for w in range(QW):
                avw = ps.tile([128, 264], F32, tag="ps", name="ps")
                for h in range(H):
                    cb = w * 256 + (h // 4) * 128
                    nc.tensor.matmul(
                        avw[:, h * 33:(h + 1) * 33],
                        lhsT=E3[h % 4][:, cb:cb + 128],
                        rhs=vns[w // 2][:, (w % 2) * 264 + h * 33:
                                        (w % 2) * 264 + (h + 1) * 33],
                        start=True, stop=True)
                av.append(avw)
            st[q]["av"] = av

        def emit_div(q):
            av = st[q]["av"]
            rc = sb.tile([128, 32], F32, tag="rc", name="rc", bufs=2)
            for w in range(QW):
                nc.scalar.copy(
                    rc[:, w * 8:(w + 1) * 8],
                    av[w][:, :].rearrange("p (x c) -> p x c", c=33)[:, :, 32])
            rc2 = sb.tile([128, 32], F32, tag="rc2", name="rc2", bufs=2)
            nc.vector.reciprocal(rc2[:], rc[:])
            # ad[w][f*64+n, h*32 + d] (token-major, bf16)
            ad = []
            for w in range(QW):
                adw = sb.tile([128, 256], BF, tag="ad", name="ad", bufs=8)
                nc.vector.tensor_mul(
                    adw[:, :].rearrange("p (h d) -> p h d", d=32),
                    av[w][:, :].rearrange("p (h c) -> p h c", c=33)[:, :, 0:32],
                    rc2[:, w * 8:(w + 1) * 8].broadcast_to((128, 8, 32)))
                ad.append(adw)
            st[q]["ad"] = ad

        def emit_tail(q):
            mr = q * QW * T * N
            ad = st[q]["ad"]
            # transpose token-major -> channel-major via matmul against I.
            pot = [ps.tile([128, 512], F32, tag="ps", name="ps")
                   for _ in range(2)]
            for cc in range(2):
                for w in range(QW):
                    nc.tensor.matmul(
                        pot[cc][:, w * 128:(w + 1) * 128],
                        lhsT=ad[w][:, cc * 128:(cc + 1) * 128],
                        rhs=idn[:],
                        start=True, stop=True)
            aT = []
            for cc in range(2):
                aTc = sb.tile([128, 512], BF, tag="aT", name="aT", bufs=4)
                if cc == 0:
                    nc.vector.tensor_copy(aTc[:], pot[cc][:])
                else:
                    nc.scalar.copy(aTc[:], pot[cc][:])
                aT.append(aTc)

            # final projection: py[b2][f*64+n, (w%2)*256 + c], f32.
            py = [ps.tile([128, 512], F32, tag="ps", name="ps")
                  for _ in range(2)]
            for w in range(QW):
                for cc in range(2):
                    nc.tensor.matmul(
                        py[w // 2][:, (w % 2) * 256:(w % 2 + 1) * 256],
                        lhsT=aT[cc][:, w * 128:(w + 1) * 128],
                        rhs=pwT[cc][:],
                        start=(cc == 0), stop=(cc == 1))
            for j in range(2):
                yo = sb.tile([128, 512], BF, tag="yo", name="yo", bufs=4)
                if j == 0:
                    nc.scalar.copy(yo[:], py[j][:])
                else:
                    nc.vector.tensor_copy(yo[:], py[j][:])
                for w2 in range(2):
                    base = mr + j * 256 + w2 * 128
                    nc.gpsimd.dma_start(
                        out=out_d[base: base + 128, :],
                        in_=yo[:, w2 * 256:(w2 + 1) * 256])

        # ---- software-pipelined main loop ----
        emit_loads(0)
        emit_proj(0)
        for i in range(nq):
            if i + 1 < nq:
                emit_loads(i + 1)
            emit_qk(i)
            emit_softmax(i)
            if i + 1 < nq:
                emit_proj(i + 1)
            emit_av(i)
            emit_div(i)
            if i > 0:
                emit_tail(i - 1)
            # drop handles no longer needed to keep memory flat
            st[i]["psS"] = None
            if i > 0:
                st[i - 1] = None
        emit_tail(nq - 1)

    nc.compile()
    return nc


def _get_prog(nwin, with_qbias):
    key = (nwin, with_qbias)
    if key not in _PROG_CACHE:
        _PROG_CACHE[key] = _build(nwin, with_qbias)
    return _PROG_CACHE[key]


def _host_prep(x, memory, q_w, q_b, kv_w, kv_b, proj_w, proj_b, rpb_table,
               nwin_core, ncores):
    """Shared host-side preprocessing: fold scales/biases, shard, cast."""
    x = np.asarray(x, np.float32)
    memory = np.asarray(memory, np.float32)
    q_w = np.asarray(q_w, np.float32)
    q_b = np.asarray(q_b, np.float32)
    kv_w = np.asarray(kv_w, np.float32)
    kv_b = np.asarray(kv_b, np.float32)
    proj_w = np.asarray(proj_w, np.float32)
    proj_b = np.asarray(proj_b, np.float32)
    rpb_table = np.asarray(rpb_table, np.float32)

    scale = np.float32(HD ** -0.5)
    qwT = np.ascontiguousarray((q_w * scale).T).astype(BF16)       # [c_in, c_out]
    kwT = np.ascontiguousarray(kv_w[:C].T).astype(BF16)
    vwT = np.ascontiguousarray(kv_w[C:].T).astype(BF16)
    pwT = np.ascontiguousarray(proj_w.T).astype(BF16)

    # logits bias, transposed per head: biasT[h][m, n]; exp() folded in.
    # Device layout [128, 2048]: row t*64+m (both t halves equal),
    # col rg*512 + w*128 + hh*64 + n, for head = rg + 4*hh (w-invariant).
    bias = rpb_table[REL_IDX.reshape(-1)].reshape(N, N, H)          # [n, m, h]
    biasT = bias.transpose(2, 1, 0)                                 # [h, m, n]
    blocks = []
    for rg in range(4):
        pair = np.concatenate([biasT[rg], biasT[rg + 4]], axis=1)   # [64, 128]
        blocks.append(np.tile(pair, (1, QW)))                       # [64, 512]
    half = np.concatenate(blocks, axis=1)                           # [64, 2048]
    expBT = np.exp(np.vstack([half, half])).astype(BF16)            # [128, 2048]

    idn = np.eye(128, dtype=BF16)

    qb_eff = q_b * scale
    with_qbias = bool(np.any(qb_eff))
    qbT = np.ascontiguousarray(qb_eff.reshape(2, 128).T).astype(np.float32)

    # k bias: constant per query in logits -> softmax-invariant, dropped.
    # v bias + proj bias: fold into a host-side output add.
    out_bias = kv_b[C:] @ proj_w.T + proj_b                          # (C,)

    # host-side transpose to channel-major (device loads these directly)
    xs = x.reshape(B_FULL * N, C).astype(BF16)
    ms = memory.reshape(B_FULL * T * N, C).astype(BF16)

    xt = nwin_core * N
    mt = nwin_core * T * N
    in_maps = []
    for c in range(ncores):
        im = {
            "xT": np.ascontiguousarray(xs[c * xt:(c + 1) * xt].T),
            "memT": np.ascontiguousarray(ms[c * mt:(c + 1) * mt].T),
            "qwT": qwT, "kwT": kwT, "vwT": vwT, "pwT": pwT,
            "expBT": expBT, "idn": idn,
        }
        if with_qbias:
            im["qbT"] = qbT
        in_maps.append(im)
    return in_maps, with_qbias, out_bias


def kernel(x, memory, q_w, q_b, kv_w, kv_b, proj_w, proj_b, rpb_table):
    in_maps, with_qbias, out_bias = _host_prep(
        x, memory, q_w, q_b, kv_w, kv_b, proj_w, proj_b, rpb_table,
        B_CORE, NCORES)
    nc = _get_prog(B_CORE, with_qbias)
    res = run_bass_kernel_spmd(nc, in_maps, list(range(NCORES)))
    out = np.concatenate([np.asarray(r["out"]) for r in res.results], axis=0)
    out = out.astype(np.float32)
    if np.any(out_bias):
        out = out + out_bias[None, :].astype(np.float32)
    return np.ascontiguousarray(out.reshape(B_FULL, T, N, C))


# revision 24
# speedup vs baseline: 1.1591x; 1.0049x over previous
"""Swin-style windowed multi-head cross-attention on 8 Trainium2 NeuronCores.

Full inputs in, full output out. Data-parallel over the window dim B_:
each of the 8 cores processes 256 windows end-to-end (no collectives).

Per-core Bass/Tile program, software-pipelined across 4-window "quads" so
the PE never idles long enough for the HAM clock-gate to re-throttle:

  iteration i emits:  loads(i+1) | QK(i) | exp+bias-mul(i) | proj(i+1) |
                      AV(i) | divide(i) | transpose+out-proj(i-1)

Algorithm per quad (4 windows, T=2 memory frames, 8 heads, hd=32):
  1. q/k projections in transposed orientation (qT/kT channel-major),
     v projection natural; v copied into persistent vn tiles with a
     per-head ones-column (33-col groups) that later yields the softmax
     denominators for free inside the AV matmul.
  2. QK^T per (window, hh, rg) into 4 PSUM banks grouped by PE row-group
     (matmuls sharing a bank must share their array row position).
  3. exp on ACT; then exp(S)*exp(B) is written MASKED into persistent
     block-diagonal E3 tiles [m, (w, hh, frame, n)]: frame-f columns only
     get rows of frame f, other rows stay zero (written once at init).
     This both applies the relative-position bias and builds the
     frame-separating block structure for AV in one DVE pass.
  4. AV: one matmul per (window, head): lhsT=E3-slice [128,128] (FWL),
     rhs=vn 33-col slice; out av[frame*64+n, (w%2)*264+h*33+{d|den}] in
     2 bf16 PSUM banks. Ones columns give denominators.
  5. reciprocal + one broadcast-divide per av bank -> ad (token-major);
     transpose back to channel-major via plain matmuls against identity
     (faster than PE transpose-mode and HAM-friendly); final projection;
     bf16 store with host-side f32 upcast.

Softmax max-subtraction is skipped: logits are ~N(0,1)+-small bias, exp is
safe in f32/bf16 and the result is mathematically identical.
q bias is applied on-device (per-partition ACT bias) only when nonzero;
k bias is a per-query constant in the logits (softmax-invariant) so it is
dropped exactly; v bias and proj bias fold into a host-side output add.
"""

import numpy as np
import ml_dtypes

import concourse.bacc as bacc
import concourse.mybir as mybir
import concourse.tile as tile
from concourse.bass_utils import run_bass_kernel_spmd

PH = PW = 8
N = 64          # tokens per window
C = 256         # channels
H = 8           # heads
HD = 32         # head dim
T = 2           # memory frames per window
NCORES = 8
B_FULL = 2048
B_CORE = B_FULL // NCORES
QW = 4          # windows per block
BF16 = ml_dtypes.bfloat16

F32 = mybir.dt.float32
BF = mybir.dt.bfloat16
AF = mybir.ActivationFunctionType


def _rel_idx(ph, pw):
    coords = np.stack(np.meshgrid(np.arange(ph), np.arange(pw), indexing="ij"))
    flat = coords.reshape(2, -1)
    rel = flat[:, :, None] - flat[:, None, :]
    rel = rel.transpose(1, 2, 0).copy()
    rel[:, :, 0] += ph - 1
    rel[:, :, 1] += pw - 1
    rel[:, :, 0] *= 2 * pw - 1
    return rel.sum(-1)  # (N, N) int


REL_IDX = _rel_idx(PH, PW)

_PROG_CACHE = {}


def _build(nwin, with_qbias):
    """Build the per-core Bass program for nwin windows."""
    assert nwin % QW == 0
    nq = nwin // QW
    xt = nwin * N            # x tokens
    mt = nwin * T * N        # memory tokens

    nc = bacc.Bacc("TRN2", target_bir_lowering=False)

    x_d = nc.dram_tensor("xT", [C, xt], BF, kind="ExternalInput").ap()
    m_d = nc.dram_tensor("memT", [C, mt], BF, kind="ExternalInput").ap()
    qw_d = nc.dram_tensor("qwT", [C, C], BF, kind="ExternalInput").ap()
    kw_d = nc.dram_tensor("kwT", [C, C], BF, kind="ExternalInput").ap()
    vw_d = nc.dram_tensor("vwT", [C, C], BF, kind="ExternalInput").ap()
    pw_d = nc.dram_tensor("pwT", [C, C], BF, kind="ExternalInput").ap()
    eb_d = nc.dram_tensor("expBT", [128, 2048], BF, kind="ExternalInput").ap()
    id_d = nc.dram_tensor("idn", [128, 128], BF, kind="ExternalInput").ap()
    if with_qbias:
        qb_d = nc.dram_tensor("qbT", [128, 2], F32, kind="ExternalInput").ap()
    out_d = nc.dram_tensor("out", [mt, C], BF, kind="ExternalOutput").ap()

    from contextlib import ExitStack
    with ExitStack() as ctx:
        tc = ctx.enter_context(tile.TileContext(nc, pool_alloc_mode="queue"))
        const = ctx.enter_context(tc.tile_pool(name="const", bufs=1))
        sb = ctx.enter_context(tc.tile_pool(name="sb", bufs=2))
        ps = ctx.enter_context(tc.tile_pool(name="ps", bufs=8, space="PSUM"))

        # ---- constants (each weight: [128, 2*C], K-chunk kc at cols kc*C) ----
        qwT_s = const.tile([128, 2 * C], BF, tag="qwT", name="qwT_s")
        kwT_s = const.tile([128, 2 * C], BF, tag="kwT", name="kwT_s")
        vwT_s = const.tile([128, 2 * C], BF, tag="vwT", name="vwT_s")
        pwT_s = const.tile([128, 2 * C], BF, tag="pwT", name="pwT_s")
        for kc in range(2):
            nc.sync.dma_start(out=qwT_s[:, kc * C:(kc + 1) * C], in_=qw_d[kc * 128:(kc + 1) * 128, :])
            nc.sync.dma_start(out=kwT_s[:, kc * C:(kc + 1) * C], in_=kw_d[kc * 128:(kc + 1) * 128, :])
            nc.sync.dma_start(out=vwT_s[:, kc * C:(kc + 1) * C], in_=vw_d[kc * 128:(kc + 1) * 128, :])
            nc.sync.dma_start(out=pwT_s[:, kc * C:(kc + 1) * C], in_=pw_d[kc * 128:(kc + 1) * 128, :])
        qwT = [qwT_s[:, 0:C], qwT_s[:, C:2 * C]]
        kwT = [kwT_s[:, 0:C], kwT_s[:, C:2 * C]]
        vwT = [vwT_s[:, 0:C], vwT_s[:, C:2 * C]]
        pwT = [pwT_s[:, 0:C], pwT_s[:, C:2 * C]]
        expB = const.tile([128, 2048], BF, tag="expB", name="expB")
        nc.sync.dma_start(out=expB[:], in_=eb_d[:, :])
        idn = const.tile([128, 128], BF, tag="idn", name="idn")
        nc.sync.dma_start(out=idn[:], in_=id_d[:, :])
        if with_qbias:
            qbT = const.tile([128, 2], F32, tag="qbT", name="qbT")
            nc.sync.dma_start(out=qbT[:], in_=qb_d[:, :])

        # persistent masked-E tiles: E3[rg][m, w*256 + hh*128 + f*64 + n]
        # = exp(S+B) if frame(m)==f else 0.  Zero blocks written once here.
        E3 = [const.tile([128, 1024], BF, tag=f"E3_{rg}", name="E3")
              for rg in range(4)]
        for e in E3:
            nc.vector.memset(e[:], 0.0)

        # persistent vn tiles with fused denominator ones-columns:
        # vn[(q%2)*2+b][m, tl*264 + h*33 + d] = v, col h*33+32 = 1.
        vn_all = [const.tile([128, 528], BF, tag=f"vn{i}", name="vn")
                  for i in range(4)]
        for v in vn_all:
            nc.vector.memset(
                v[:, :].rearrange("p (x c) -> p x c", c=33)[:, :, 32:33], 1.0)

        st = [{} for _ in range(nq)]  # per-quad tile handles

        def emit_loads(q):
            xr = q * QW * N
            mr = q * QW * T * N
            xT = sb.tile([128, 512], BF, tag="xT", name="xT", bufs=4)
            for cc in range(2):
                nc.sync.dma_start(out=xT[:, cc * 256:(cc + 1) * 256],
                                  in_=x_d[cc * 128:(cc + 1) * 128, xr:xr + 256])
            memT = []
            for cc in range(2):
                mT = sb.tile([128, 512], BF, tag="memT", name="memT", bufs=8)
                nc.sync.dma_start(out=mT[:], in_=m_d[cc * 128:(cc + 1) * 128, mr:mr + 512])
                memT.append(mT)
            st[q]["xT"] = xT
            st[q]["memT"] = memT

        def emit_proj(q):
            xT = st[q]["xT"]
            memT = st[q]["memT"]
            # ---- q projection (transposed: qT[c_out%128, hh*256 + w*64+n]) ----
            pq = ps.tile([128, 512], F32, tag="ps", name="ps")
            for mo in range(2):
                for kc in range(2):
                    nc.tensor.matmul(
                        pq[:, mo * 256:(mo + 1) * 256],
                        lhsT=qwT[kc][:, mo * 128:(mo + 1) * 128],
                        rhs=xT[:, kc * 256:(kc + 1) * 256],
                        start=(kc == 0), stop=(kc == 1))
            qT = sb.tile([128, 512], BF, tag="qT", name="qT", bufs=4)
            if with_qbias:
                for mo in range(2):
                    nc.scalar.activation(qT[:, mo * 256:(mo + 1) * 256],
                                         pq[:, mo * 256:(mo + 1) * 256],
                                         AF.Copy, bias=qbT[:, mo:mo + 1])
            else:
                nc.scalar.copy(qT[:], pq[:])
            st[q]["qT"] = qT

            # ---- k projection (transposed: kT[mo][c, w*128 + t*64+n]) ----
            kT = []
            for mo in range(2):
                pk = ps.tile([128, 512], F32, tag="ps", name="ps")
                for kc in range(2):
                    nc.tensor.matmul(
                        pk[:],
                        lhsT=kwT[kc][:, mo * 128:(mo + 1) * 128],
                        rhs=memT[kc][:],
                        start=(kc == 0), stop=(kc == 1))
                kt = sb.tile([128, 512], BF, tag="kT", name="kT", bufs=6)
                if mo == 0:
                    nc.scalar.copy(kt[:], pk[:])
                else:
                    nc.vector.tensor_copy(kt[:], pk[:])
                kT.append(kt)
            st[q]["kT"] = kT

            # ---- v projection (natural) -> persistent vn (33-col groups) ----
            vns = []
            for b in range(2):
                pv = ps.tile([128, 512], F32, tag="ps", name="ps")
                for tl in range(2):
                    tt = b * 2 + tl
                    for kc in range(2):
                        nc.tensor.matmul(
                            pv[:, tl * 256:(tl + 1) * 256],
                            lhsT=memT[kc][:, tt * 128:(tt + 1) * 128],
                            rhs=vwT[kc][:],
                            start=(kc == 0), stop=(kc == 1))
                vt = vn_all[(q % 2) * 2 + b]
                nc.scalar.copy(
                    vt[:, :].rearrange("p (tl h c) -> p tl h c", tl=2, c=33)
                        [:, :, :, 0:32],
                    pv[:, :].rearrange("p (tl h d) -> p tl h d", tl=2, d=32))
                vns.append(vt)
            st[q]["vn"] = vns

        def emit_qk(q):
            qT = st[q]["qT"]
            kT = st[q]["kT"]
            # Bank rg holds heads {rg, rg+4}: psS[rg][m, w*128 + hh*64 + n].
            psS = [ps.tile([128, 512], F32, tag="ps", name="ps") for _ in range(4)]
            for hh in range(2):
                for w in range(QW):
                    for rg in range(4):
                        nc.tensor.matmul(
                            psS[rg][:, w * 128 + hh * 64: w * 128 + (hh + 1) * 64],
                            lhsT=kT[hh][rg * 32:(rg + 1) * 32, w * 128:(w + 1) * 128],
                            rhs=qT[rg * 32:(rg + 1) * 32,
                                   hh * 256 + w * 64: hh * 256 + (w + 1) * 64],
                            start=True, stop=True,
                            tile_position=(rg * 32, 0))
            st[q]["psS"] = psS

        def emit_softmax(q):
            # exp on ACT into one wide E tile, then two masked bias-multiply
            # ops write the block-diagonal E3 (g = 32 (rg,w,hh) blocks).
            psS = st[q]["psS"]
            for rg in range(4):
                Ew = sb.tile([128, 512], BF, tag="E", name="E", bufs=8)
                nc.scalar.activation(Ew[:], psS[rg][:], AF.Exp)
                for f in range(2):
                    rows = slice(f * 64, (f + 1) * 64)
                    nc.vector.tensor_mul(
                        E3[rg][rows, :].rearrange(
                            "p (w hh f2 n) -> p w hh f2 n", w=4, hh=2, f2=2)
                            [:, :, :, f, :],
                        Ew[rows, :].rearrange(
                            "p (w hh n) -> p w hh n", w=4, hh=2),
                        expB[rows, rg * 512:(rg + 1) * 512].rearrange(
                            "p (w hh n) -> p w hh n", w=4, hh=2))

        def emit_av(q):
            vns = st[q]["vn"]
            # av[w][f*64+n, h*33 + {d|den}], f32, one bank per window.
            av = []
            for w in range(QW):
                avw = ps.tile([128, 264], F32, tag="ps", name="ps")
                for h in range(H):
                    cb = w * 256 + (h // 4) * 128
                    nc.tensor.matmul(
                        avw[:, h * 33:(h + 1) * 33],
                        lhsT=E3[h % 4][:, cb:cb + 128],
                        rhs=vns[w // 2][:, (w % 2) * 264 + h * 33:
                                        (w % 2) * 264 + (h + 1) * 33],
                        start=True, stop=True)
                av.append(avw)
            st[q]["av"] = av

        def emit_div(q):
            av = st[q]["av"]
            rc = sb.tile([128, 32], F32, tag="rc", name="rc", bufs=2)
            for w in range(QW):
                nc.scalar.copy(
                    rc[:, w * 8:(w + 1) * 8],
                    av[w][:, :].rearrange("p (x c) -> p x c", c=33)[:, :, 32])
            rc2 = sb.tile([128, 32], F32, tag="rc2", name="rc2", bufs=2)
            nc.vector.reciprocal(rc2[:], rc[:])
            # ad[w][f*64+n, h*32 + d] (token-major, bf16)
            ad = []
            for w in range(QW):
                adw = sb.tile([128, 256], BF, tag="ad", name="ad", bufs=8)
                nc.vector.tensor_mul(
                    adw[:, :].rearrange("p (h d) -> p h d", d=32),
                    av[w][:, :].rearrange("p (h c) -> p h c", c=33)[:, :, 0:32],
                    rc2[:, w * 8:(w + 1) * 8].broadcast_to((128, 8, 32)))
                ad.append(adw)
            st[q]["ad"] = ad

        def emit_tail(q):
            mr = q * QW * T * N
            ad = st[q]["ad"]
            # transpose token-major -> channel-major via matmul against I.
            pot = [ps.tile([128, 512], F32, tag="ps", name="ps")
                   for _ in range(2)]
            for cc in range(2):
                for w in range(QW):
                    nc.tensor.matmul(
                        pot[cc][:, w * 128:(w + 1) * 128],
                        lhsT=ad[w][:, cc * 128:(cc + 1) * 128],
                        rhs=idn[:],
                        start=True, stop=True)
            aT = []
            for cc in range(2):
                aTc = sb.tile([128, 512], BF, tag="aT", name="aT", bufs=6)
                if cc == 0:
                    nc.vector.tensor_copy(aTc[:], pot[cc][:])
                else:
                    nc.scalar.copy(aTc[:], pot[cc][:])
                aT.append(aTc)

            # final projection: py[b2][f*64+n, (w%2)*256 + c], f32.
            py = [ps.tile([128, 512], F32, tag="ps", name="ps")
                  for _ in range(2)]
            for w in range(QW):
                for cc in range(2):
                    nc.tensor.matmul(
                        py[w // 2][:, (w % 2) * 256:(w % 2 + 1) * 256],
                        lhsT=aT[cc][:, w * 128:(w + 1) * 128],
                        rhs=pwT[cc][:],
                        start=(cc == 0), stop=(cc == 1))
            for j in range(2):
                yo = sb.tile([128, 512], BF, tag="yo", name="yo", bufs=6)
                if j == 0:
                    nc.scalar.copy(yo[:], py[j][:])
                else:
                    nc.vector.tensor_copy(yo[:], py[j][:])
                for w2 in range(2):
                    base = mr + j * 256 + w2 * 128
                    nc.gpsimd.dma_start(
                        out=out_d[base: base + 128, :],
                        in_=yo[:, w2 * 256:(w2 + 1) * 256])

        # ---- software-pipelined main loop ----
        emit_loads(0)
        emit_proj(0)
        for i in range(nq):
            if i + 1 < nq:
                emit_loads(i + 1)
            emit_qk(i)
            emit_softmax(i)
            if i + 1 < nq:
                emit_proj(i + 1)
            emit_av(i)
            emit_div(i)
            if i > 0:
                emit_tail(i - 1)
            # drop handles no longer needed to keep memory flat
            st[i]["psS"] = None
            if i > 0:
                st[i - 1] = None
        emit_tail(nq - 1)

    nc.compile()
    return nc


def _get_prog(nwin, with_qbias):
    key = (nwin, with_qbias)
    if key not in _PROG_CACHE:
        _PROG_CACHE[key] = _build(nwin, with_qbias)
    return _PROG_CACHE[key]


def _host_prep(x, memory, q_w, q_b, kv_w, kv_b, proj_w, proj_b, rpb_table,
               nwin_core, ncores):
    """Shared host-side preprocessing: fold scales/biases, shard, cast."""
    x = np.asarray(x, np.float32)
    memory = np.asarray(memory, np.float32)
    q_w = np.asarray(q_w, np.float32)
    q_b = np.asarray(q_b, np.float32)
    kv_w = np.asarray(kv_w, np.float32)
    kv_b = np.asarray(kv_b, np.float32)
    proj_w = np.asarray(proj_w, np.float32)
    proj_b = np.asarray(proj_b, np.float32)
    rpb_table = np.asarray(rpb_table, np.float32)

    scale = np.float32(HD ** -0.5)
    qwT = np.ascontiguousarray((q_w * scale).T).astype(BF16)       # [c_in, c_out]
    kwT = np.ascontiguousarray(kv_w[:C].T).astype(BF16)
    vwT = np.ascontiguousarray(kv_w[C:].T).astype(BF16)
    pwT = np.ascontiguousarray(proj_w.T).astype(BF16)

    # logits bias, transposed per head: biasT[h][m, n]; exp() folded in.
    # Device layout [128, 2048]: row t*64+m (both t halves equal),
    # col rg*512 + w*128 + hh*64 + n, for head = rg + 4*hh (w-invariant).
    bias = rpb_table[REL_IDX.reshape(-1)].reshape(N, N, H)          # [n, m, h]
    biasT = bias.transpose(2, 1, 0)                                 # [h, m, n]
    blocks = []
    for rg in range(4):
        pair = np.concatenate([biasT[rg], biasT[rg + 4]], axis=1)   # [64, 128]
        blocks.append(np.tile(pair, (1, QW)))                       # [64, 512]
    half = np.concatenate(blocks, axis=1)                           # [64, 2048]
    expBT = np.exp(np.vstack([half, half])).astype(BF16)            # [128, 2048]

    idn = np.eye(128, dtype=BF16)

    qb_eff = q_b * scale
    with_qbias = bool(np.any(qb_eff))
    qbT = np.ascontiguousarray(qb_eff.reshape(2, 128).T).astype(np.float32)

    # k bias: constant per query in logits -> softmax-invariant, dropped.
    # v bias + proj bias: fold into a host-side output add.
    out_bias = kv_b[C:] @ proj_w.T + proj_b                          # (C,)

    # host-side transpose to channel-major (device loads these directly)
    xs = x.reshape(B_FULL * N, C).astype(BF16)
    ms = memory.reshape(B_FULL * T * N, C).astype(BF16)

    xt = nwin_core * N
    mt = nwin_core * T * N
    in_maps = []
    for c in range(ncores):
        im = {
            "xT": np.ascontiguousarray(xs[c * xt:(c + 1) * xt].T),
            "memT": np.ascontiguousarray(ms[c * mt:(c + 1) * mt].T),
            "qwT": qwT, "kwT": kwT, "vwT": vwT, "pwT": pwT,
            "expBT": expBT, "idn": idn,
        }
        if with_qbias:
            im["qbT"] = qbT
        in_maps.append(im)
    return in_maps, with_qbias, out_bias


def kernel(x, memory, q_w, q_b, kv_w, kv_b, proj_w, proj_b, rpb_table):
    in_maps, with_qbias, out_bias = _host_prep(
        x, memory, q_w, q_b, kv_w, kv_b, proj_w, proj_b, rpb_table,
        B_CORE, NCORES)
    nc = _get_prog(B_CORE, with_qbias)
    res = run_bass_kernel_spmd(nc, in_maps, list(range(NCORES)))
    out = np.concatenate([np.asarray(r["out"]) for r in res.results], axis=0)
    out = out.astype(np.float32)
    if np.any(out_bias):
        out = out + out_bias[None, :].astype(np.float32)
    return np.ascontiguousarray(out.reshape(B_FULL, T, N, C))


# revision 25
# speedup vs baseline: 1.1629x; 1.0033x over previous
"""Swin-style windowed multi-head cross-attention on 8 Trainium2 NeuronCores.

Full inputs in, full output out. Data-parallel over the window dim B_:
each of the 8 cores processes 256 windows end-to-end (no collectives).

Per-core Bass/Tile program, software-pipelined across 4-window "quads" so
the PE never idles long enough for the HAM clock-gate to re-throttle:

  iteration i emits:  loads(i+1) | QK(i) | exp+bias-mul(i) | proj(i+1) |
                      AV(i) | divide(i) | transpose+out-proj(i-1)

Algorithm per quad (4 windows, T=2 memory frames, 8 heads, hd=32):
  1. q/k projections in transposed orientation (qT/kT channel-major),
     v projection natural; v copied into persistent vn tiles with a
     per-head ones-column (33-col groups) that later yields the softmax
     denominators for free inside the AV matmul.
  2. QK^T per (window, hh, rg) into 4 PSUM banks grouped by PE row-group
     (matmuls sharing a bank must share their array row position).
  3. exp on ACT; then exp(S)*exp(B) is written MASKED into persistent
     block-diagonal E3 tiles [m, (w, hh, frame, n)]: frame-f columns only
     get rows of frame f, other rows stay zero (written once at init).
     This both applies the relative-position bias and builds the
     frame-separating block structure for AV in one DVE pass.
  4. AV: one matmul per (window, head): lhsT=E3-slice [128,128] (FWL),
     rhs=vn 33-col slice; out av[frame*64+n, (w%2)*264+h*33+{d|den}] in
     2 bf16 PSUM banks. Ones columns give denominators.
  5. reciprocal + one broadcast-divide per av bank -> ad (token-major);
     transpose back to channel-major via plain matmuls against identity
     (faster than PE transpose-mode and HAM-friendly); final projection;
     bf16 store with host-side f32 upcast.

Softmax max-subtraction is skipped: logits are ~N(0,1)+-small bias, exp is
safe in f32/bf16 and the result is mathematically identical.
q bias is applied on-device (per-partition ACT bias) only when nonzero;
k bias is a per-query constant in the logits (softmax-invariant) so it is
dropped exactly; v bias and proj bias fold into a host-side output add.
"""

import numpy as np
import ml_dtypes

import concourse.bacc as bacc
import concourse.mybir as mybir
import concourse.tile as tile
from concourse.bass_utils import run_bass_kernel_spmd

PH = PW = 8
N = 64          # tokens per window
C = 256         # channels
H = 8           # heads
HD = 32         # head dim
T = 2           # memory frames per window
NCORES = 8
B_FULL = 2048
B_CORE = B_FULL // NCORES
QW = 4          # windows per block
BF16 = ml_dtypes.bfloat16

F32 = mybir.dt.float32
BF = mybir.dt.bfloat16
AF = mybir.ActivationFunctionType


def _rel_idx(ph, pw):
    coords = np.stack(np.meshgrid(np.arange(ph), np.arange(pw), indexing="ij"))
    flat = coords.reshape(2, -1)
    rel = flat[:, :, None] - flat[:, None, :]
    rel = rel.transpose(1, 2, 0).copy()
    rel[:, :, 0] += ph - 1
    rel[:, :, 1] += pw - 1
    rel[:, :, 0] *= 2 * pw - 1
    return rel.sum(-1)  # (N, N) int


REL_IDX = _rel_idx(PH, PW)

_PROG_CACHE = {}


def _build(nwin, with_qbias):
    """Build the per-core Bass program for nwin windows."""
    assert nwin % QW == 0
    nq = nwin // QW
    xt = nwin * N            # x tokens
    mt = nwin * T * N        # memory tokens

    nc = bacc.Bacc("TRN2", target_bir_lowering=False)

    x_d = nc.dram_tensor("xT", [C, xt], BF, kind="ExternalInput").ap()
    m_d = nc.dram_tensor("memT", [C, mt], BF, kind="ExternalInput").ap()
    qw_d = nc.dram_tensor("qwT", [C, C], BF, kind="ExternalInput").ap()
    kw_d = nc.dram_tensor("kwT", [C, C], BF, kind="ExternalInput").ap()
    vw_d = nc.dram_tensor("vwT", [C, C], BF, kind="ExternalInput").ap()
    pw_d = nc.dram_tensor("pwT", [C, C], BF, kind="ExternalInput").ap()
    eb_d = nc.dram_tensor("expBT", [128, 2048], BF, kind="ExternalInput").ap()
    id_d = nc.dram_tensor("idn", [128, 128], BF, kind="ExternalInput").ap()
    if with_qbias:
        qb_d = nc.dram_tensor("qbT", [128, 2], F32, kind="ExternalInput").ap()
    out_d = nc.dram_tensor("out", [mt, C], BF, kind="ExternalOutput").ap()

    from contextlib import ExitStack
    with ExitStack() as ctx:
        tc = ctx.enter_context(tile.TileContext(nc, pool_alloc_mode="queue"))
        const = ctx.enter_context(tc.tile_pool(name="const", bufs=1))
        sb = ctx.enter_context(tc.tile_pool(name="sb", bufs=2))
        ps = ctx.enter_context(tc.tile_pool(name="ps", bufs=8, space="PSUM"))

        # ---- constants (each weight: [128, 2*C], K-chunk kc at cols kc*C) ----
        qwT_s = const.tile([128, 2 * C], BF, tag="qwT", name="qwT_s")
        kwT_s = const.tile([128, 2 * C], BF, tag="kwT", name="kwT_s")
        vwT_s = const.tile([128, 2 * C], BF, tag="vwT", name="vwT_s")
        pwT_s = const.tile([128, 2 * C], BF, tag="pwT", name="pwT_s")
        for kc in range(2):
            nc.sync.dma_start(out=qwT_s[:, kc * C:(kc + 1) * C], in_=qw_d[kc * 128:(kc + 1) * 128, :])
            nc.sync.dma_start(out=kwT_s[:, kc * C:(kc + 1) * C], in_=kw_d[kc * 128:(kc + 1) * 128, :])
            nc.sync.dma_start(out=vwT_s[:, kc * C:(kc + 1) * C], in_=vw_d[kc * 128:(kc + 1) * 128, :])
            nc.sync.dma_start(out=pwT_s[:, kc * C:(kc + 1) * C], in_=pw_d[kc * 128:(kc + 1) * 128, :])
        qwT = [qwT_s[:, 0:C], qwT_s[:, C:2 * C]]
        kwT = [kwT_s[:, 0:C], kwT_s[:, C:2 * C]]
        vwT = [vwT_s[:, 0:C], vwT_s[:, C:2 * C]]
        pwT = [pwT_s[:, 0:C], pwT_s[:, C:2 * C]]
        expB = const.tile([128, 2048], BF, tag="expB", name="expB")
        nc.sync.dma_start(out=expB[:], in_=eb_d[:, :])
        idn = const.tile([128, 128], BF, tag="idn", name="idn")
        nc.sync.dma_start(out=idn[:], in_=id_d[:, :])
        if with_qbias:
            qbT = const.tile([128, 2], F32, tag="qbT", name="qbT")
            nc.sync.dma_start(out=qbT[:], in_=qb_d[:, :])

        # persistent masked-E tiles: E3[rg][m, w*256 + hh*128 + f*64 + n]
        # = exp(S+B) if frame(m)==f else 0.  Zero blocks written once here.
        E3 = [const.tile([128, 1024], BF, tag=f"E3_{rg}", name="E3")
              for rg in range(8)]
        for e in E3:
            nc.vector.memset(e[:], 0.0)

        # persistent vn tiles with fused denominator ones-columns:
        # vn[(q%2)*2+b][m, tl*264 + h*33 + d] = v, col h*33+32 = 1.
        vn_all = [const.tile([128, 528], BF, tag=f"vn{i}", name="vn")
                  for i in range(8)]
        for v in vn_all:
            nc.vector.memset(
                v[:, :].rearrange("p (x c) -> p x c", c=33)[:, :, 32:33], 1.0)

        st = [{} for _ in range(nq)]  # per-quad tile handles

        def emit_loads(q):
            xr = q * QW * N
            mr = q * QW * T * N
            xT = sb.tile([128, 512], BF, tag="xT", name="xT", bufs=4)
            for cc in range(2):
                nc.sync.dma_start(out=xT[:, cc * 256:(cc + 1) * 256],
                                  in_=x_d[cc * 128:(cc + 1) * 128, xr:xr + 256])
            memT = []
            for cc in range(2):
                mT = sb.tile([128, 512], BF, tag="memT", name="memT", bufs=8)
                nc.sync.dma_start(out=mT[:], in_=m_d[cc * 128:(cc + 1) * 128, mr:mr + 512])
                memT.append(mT)
            st[q]["xT"] = xT
            st[q]["memT"] = memT

        def emit_proj(q):
            xT = st[q]["xT"]
            memT = st[q]["memT"]
            # ---- q projection (transposed: qT[c_out%128, hh*256 + w*64+n]) ----
            pq = ps.tile([128, 512], F32, tag="ps", name="ps")
            for mo in range(2):
                for kc in range(2):
                    nc.tensor.matmul(
                        pq[:, mo * 256:(mo + 1) * 256],
                        lhsT=qwT[kc][:, mo * 128:(mo + 1) * 128],
                        rhs=xT[:, kc * 256:(kc + 1) * 256],
                        start=(kc == 0), stop=(kc == 1))
            qT = sb.tile([128, 512], BF, tag="qT", name="qT", bufs=4)
            if with_qbias:
                for mo in range(2):
                    nc.scalar.activation(qT[:, mo * 256:(mo + 1) * 256],
                                         pq[:, mo * 256:(mo + 1) * 256],
                                         AF.Copy, bias=qbT[:, mo:mo + 1])
            else:
                nc.scalar.copy(qT[:], pq[:])
            st[q]["qT"] = qT

            # ---- k projection (transposed: kT[mo][c, w*128 + t*64+n]) ----
            kT = []
            for mo in range(2):
                pk = ps.tile([128, 512], F32, tag="ps", name="ps")
                for kc in range(2):
                    nc.tensor.matmul(
                        pk[:],
                        lhsT=kwT[kc][:, mo * 128:(mo + 1) * 128],
                        rhs=memT[kc][:],
                        start=(kc == 0), stop=(kc == 1))
                kt = sb.tile([128, 512], BF, tag="kT", name="kT", bufs=6)
                if mo == 0:
                    nc.scalar.copy(kt[:], pk[:])
                else:
                    nc.vector.tensor_copy(kt[:], pk[:])
                kT.append(kt)
            st[q]["kT"] = kT

            # ---- v projection (natural) -> persistent vn (33-col groups) ----
            vns = []
            for b in range(2):
                pv = ps.tile([128, 512], F32, tag="ps", name="ps")
                for tl in range(2):
                    tt = b * 2 + tl
                    for kc in range(2):
                        nc.tensor.matmul(
                            pv[:, tl * 256:(tl + 1) * 256],
                            lhsT=memT[kc][:, tt * 128:(tt + 1) * 128],
                            rhs=vwT[kc][:],
                            start=(kc == 0), stop=(kc == 1))
                vt = vn_all[(q % 4) * 2 + b]
                nc.scalar.copy(
                    vt[:, :].rearrange("p (tl h c) -> p tl h c", tl=2, c=33)
                        [:, :, :, 0:32],
                    pv[:, :].rearrange("p (tl h d) -> p tl h d", tl=2, d=32))
                vns.append(vt)
            st[q]["vn"] = vns

        def emit_qk(q):
            qT = st[q]["qT"]
            kT = st[q]["kT"]
            # Bank rg holds heads {rg, rg+4}: psS[rg][m, w*128 + hh*64 + n].
            psS = [ps.tile([128, 512], F32, tag="ps", name="ps") for _ in range(4)]
            for hh in range(2):
                for w in range(QW):
                    for rg in range(4):
                        nc.tensor.matmul(
                            psS[rg][:, w * 128 + hh * 64: w * 128 + (hh + 1) * 64],
                            lhsT=kT[hh][rg * 32:(rg + 1) * 32, w * 128:(w + 1) * 128],
                            rhs=qT[rg * 32:(rg + 1) * 32,
                                   hh * 256 + w * 64: hh * 256 + (w + 1) * 64],
                            start=True, stop=True,
                            tile_position=(rg * 32, 0))
            st[q]["psS"] = psS

        def emit_softmax(q):
            # exp on ACT into one wide E tile, then two masked bias-multiply
            # ops write the block-diagonal E3 (g = 32 (rg,w,hh) blocks).
            psS = st[q]["psS"]
            for rg in range(4):
                Ew = sb.tile([128, 512], BF, tag="E", name="E", bufs=8)
                nc.scalar.activation(Ew[:], psS[rg][:], AF.Exp)
                for f in range(2):
                    rows = slice(f * 64, (f + 1) * 64)
                    nc.vector.tensor_mul(
                        E3[(q % 2) * 4 + rg][rows, :].rearrange(
                            "p (w hh f2 n) -> p w hh f2 n", w=4, hh=2, f2=2)
                            [:, :, :, f, :],
                        Ew[rows, :].rearrange(
                            "p (w hh n) -> p w hh n", w=4, hh=2),
                        expB[rows, rg * 512:(rg + 1) * 512].rearrange(
                            "p (w hh n) -> p w hh n", w=4, hh=2))

        def emit_av(q):
            vns = st[q]["vn"]
            # av[w][f*64+n, h*33 + {d|den}], f32, one bank per window.
            av = []
            for w in range(QW):
                avw = ps.tile([128, 264], F32, tag="ps", name="ps")
                for h in range(H):
                    cb = w * 256 + (h // 4) * 128
                    nc.tensor.matmul(
                        avw[:, h * 33:(h + 1) * 33],
                        lhsT=E3[(q % 2) * 4 + h % 4][:, cb:cb + 128],
                        rhs=vns[w // 2][:, (w % 2) * 264 + h * 33:
                                        (w % 2) * 264 + (h + 1) * 33],
                        start=True, stop=True)
                av.append(avw)
            st[q]["av"] = av

        def emit_div(q):
            av = st[q]["av"]
            rc = sb.tile([128, 32], F32, tag="rc", name="rc", bufs=2)
            for w in range(QW):
                nc.scalar.copy(
                    rc[:, w * 8:(w + 1) * 8],
                    av[w][:, :].rearrange("p (x c) -> p x c", c=33)[:, :, 32])
            rc2 = sb.tile([128, 32], F32, tag="rc2", name="rc2", bufs=2)
            nc.vector.reciprocal(rc2[:], rc[:])
            # ad[w][f*64+n, h*32 + d] (token-major, bf16)
            ad = []
            for w in range(QW):
                adw = sb.tile([128, 256], BF, tag="ad", name="ad", bufs=8)
                nc.vector.tensor_mul(
                    adw[:, :].rearrange("p (h d) -> p h d", d=32),
                    av[w][:, :].rearrange("p (h c) -> p h c", c=33)[:, :, 0:32],
                    rc2[:, w * 8:(w + 1) * 8].broadcast_to((128, 8, 32)))
                ad.append(adw)
            st[q]["ad"] = ad

        def emit_tail(q):
            mr = q * QW * T * N
            ad = st[q]["ad"]
            # transpose token-major -> channel-major via matmul against I.
            pot = [ps.tile([128, 512], F32, tag="ps", name="ps")
                   for _ in range(2)]
            for cc in range(2):
                for w in range(QW):
                    nc.tensor.matmul(
                        pot[cc][:, w * 128:(w + 1) * 128],
                        lhsT=ad[w][:, cc * 128:(cc + 1) * 128],
                        rhs=idn[:],
                        start=True, stop=True)
            aT = []
            for cc in range(2):
                aTc = sb.tile([128, 512], BF, tag="aT", name="aT", bufs=6)
                if cc == 0:
                    nc.vector.tensor_copy(aTc[:], pot[cc][:])
                else:
                    nc.scalar.copy(aTc[:], pot[cc][:])
                aT.append(aTc)

            # final projection: py[b2][f*64+n, (w%2)*256 + c], f32.
            py = [ps.tile([128, 512], F32, tag="ps", name="ps")
                  for _ in range(2)]
            for w in range(QW):
                for cc in range(2):
                    nc.tensor.matmul(
                        py[w // 2][:, (w % 2) * 256:(w % 2 + 1) * 256],
                        lhsT=aT[cc][:, w * 128:(w + 1) * 128],
                        rhs=pwT[cc][:],
                        start=(cc == 0), stop=(cc == 1))
            for j in range(2):
                yo = sb.tile([128, 512], BF, tag="yo", name="yo", bufs=6)
                if j == 0:
                    nc.scalar.copy(yo[:], py[j][:])
                else:
                    nc.vector.tensor_copy(yo[:], py[j][:])
                for w2 in range(2):
                    base = mr + j * 256 + w2 * 128
                    nc.gpsimd.dma_start(
                        out=out_d[base: base + 128, :],
                        in_=yo[:, w2 * 256:(w2 + 1) * 256])

        # ---- software-pipelined main loop ----
        emit_loads(0)
        emit_proj(0)
        for i in range(nq):
            if i + 1 < nq:
                emit_loads(i + 1)
            emit_qk(i)
            emit_softmax(i)
            if i + 1 < nq:
                emit_proj(i + 1)
            emit_av(i)
            emit_div(i)
            if i > 0:
                emit_tail(i - 1)
            # drop handles no longer needed to keep memory flat
            st[i]["psS"] = None
            if i > 0:
                st[i - 1] = None
        emit_tail(nq - 1)

    nc.compile()
    return nc


def _get_prog(nwin, with_qbias):
    key = (nwin, with_qbias)
    if key not in _PROG_CACHE:
        _PROG_CACHE[key] = _build(nwin, with_qbias)
    return _PROG_CACHE[key]


def _host_prep(x, memory, q_w, q_b, kv_w, kv_b, proj_w, proj_b, rpb_table,
               nwin_core, ncores):
    """Shared host-side preprocessing: fold scales/biases, shard, cast."""
    x = np.asarray(x, np.float32)
    memory = np.asarray(memory, np.float32)
    q_w = np.asarray(q_w, np.float32)
    q_b = np.asarray(q_b, np.float32)
    kv_w = np.asarray(kv_w, np.float32)
    kv_b = np.asarray(kv_b, np.float32)
    proj_w = np.asarray(proj_w, np.float32)
    proj_b = np.asarray(proj_b, np.float32)
    rpb_table = np.asarray(rpb_table, np.float32)

    scale = np.float32(HD ** -0.5)
    qwT = np.ascontiguousarray((q_w * scale).T).astype(BF16)       # [c_in, c_out]
    kwT = np.ascontiguousarray(kv_w[:C].T).astype(BF16)
    vwT = np.ascontiguousarray(kv_w[C:].T).astype(BF16)
    pwT = np.ascontiguousarray(proj_w.T).astype(BF16)

    # logits bias, transposed per head: biasT[h][m, n]; exp() folded in.
    # Device layout [128, 2048]: row t*64+m (both t halves equal),
    # col rg*512 + w*128 + hh*64 + n, for head = rg + 4*hh (w-invariant).
    bias = rpb_table[REL_IDX.reshape(-1)].reshape(N, N, H)          # [n, m, h]
    biasT = bias.transpose(2, 1, 0)                                 # [h, m, n]
    blocks = []
    for rg in range(4):
        pair = np.concatenate([biasT[rg], biasT[rg + 4]], axis=1)   # [64, 128]
        blocks.append(np.tile(pair, (1, QW)))                       # [64, 512]
    half = np.concatenate(blocks, axis=1)                           # [64, 2048]
    expBT = np.exp(np.vstack([half, half])).astype(BF16)            # [128, 2048]

    idn = np.eye(128, dtype=BF16)

    qb_eff = q_b * scale
    with_qbias = bool(np.any(qb_eff))
    qbT = np.ascontiguousarray(qb_eff.reshape(2, 128).T).astype(np.float32)

    # k bias: constant per query in logits -> softmax-invariant, dropped.
    # v bias + proj bias: fold into a host-side output add.
    out_bias = kv_b[C:] @ proj_w.T + proj_b                          # (C,)

    # host-side transpose to channel-major (device loads these directly)
    xs = x.reshape(B_FULL * N, C).astype(BF16)
    ms = memory.reshape(B_FULL * T * N, C).astype(BF16)

    xt = nwin_core * N
    mt = nwin_core * T * N
    in_maps = []
    for c in range(ncores):
        im = {
            "xT": np.ascontiguousarray(xs[c * xt:(c + 1) * xt].T),
            "memT": np.ascontiguousarray(ms[c * mt:(c + 1) * mt].T),
            "qwT": qwT, "kwT": kwT, "vwT": vwT, "pwT": pwT,
            "expBT": expBT, "idn": idn,
        }
        if with_qbias:
            im["qbT"] = qbT
        in_maps.append(im)
    return in_maps, with_qbias, out_bias


def kernel(x, memory, q_w, q_b, kv_w, kv_b, proj_w, proj_b, rpb_table):
    in_maps, with_qbias, out_bias = _host_prep(
        x, memory, q_w, q_b, kv_w, kv_b, proj_w, proj_b, rpb_table,
        B_CORE, NCORES)
    nc = _get_prog(B_CORE, with_qbias)
    res = run_bass_kernel_spmd(nc, in_maps, list(range(NCORES)))
    out = np.concatenate([np.asarray(r["out"]) for r in res.results], axis=0)
    out = out.astype(np.float32)
    if np.any(out_bias):
        out = out + out_bias[None, :].astype(np.float32)
    return np.ascontiguousarray(out.reshape(B_FULL, T, N, C))
